# revision 8
# baseline (speedup 1.0000x reference)
"""GCN layer (linear + weighted scatter-add aggregation) on 8 TRN2 NeuronCores.

Reference computation:
    h = x @ W.T                      [N, D]
    out[r] = sum_{e: row[e]==r} val[e] * h[col[e]]

Key identity: the linear layer commutes past the (linear) aggregation:
    out = (A @ x) @ W.T    where A[r,c] = sum of val over edges (r,c)
so we aggregate raw x rows first (8x less matmul work, no h materialization).

Distribution: destination nodes are sharded 12500/core (edges partitioned by
destination so the segment-sum is fully local; x is replicated to each core's
HBM by the host, so no collective is needed).

Per-core algorithm ("perm-pack"):
  - Host packs *whole* destinations into "windows" of <=32 dests and <=1024
    edge slots (avg degree 32 makes both caps bind together, ~3% padding).
    The packing order defines a per-core virtual destination numbering;
    output rows are inverse-permuted (and summed, if a dest was split) on
    the host.
  - A window is 8 "groups" of 128 edge slots. One "call" = 4 windows = 32
    groups = 4096 slots: one indirect DMA gathers 4096 x rows (bf16, 512B
    each) into SBUF as [128 slots, 32 groups, 256]. A banded scaled one-hot
    S ([128, 32] per group: S[p,r] = val[p] * (rank[p]==r)) is built with 2
    batched DVE ops per call.
  - PE: per group one matmul (lhsT = S [128,32], rhs = gathered rows
    [128,256]) accumulating over the window's 8 groups into a 32-partition
    PSUM region; 4 windows fill one [128,256] f32 PSUM tile = 128 virtual
    dests' aggregate. The aggregate is cast to bf16, transposed on PE
    (identity trick), and multiplied by W.T (bf16, f32 PSUM) into the output
    block. No scatter, no atomics, no collectives.
"""

import os
import sys

sys.path.insert(0, "/opt/trn_rl_repo")
os.environ.setdefault("MYCRO_LOCAL_CACHE", "1")

from contextlib import ExitStack

import numpy as np
import ml_dtypes

import concourse.bass as bass
import concourse.bacc as bacc
import concourse.mybir as mybir
import concourse.tile as tile
from concourse.bass_utils import run_bass_kernel_spmd

N_NODES = 100000
N_CORES = 8
NPC = N_NODES // N_CORES  # dests per core
D = 256
SLOTS = 128  # edge slots per group (= matmul K)
W_RANK = 32  # dests per window (= matmul M)
W_GROUPS = 8  # groups per window
W_SLOTS = W_GROUPS * SLOTS  # 1024 edge slots per window
CG = 32  # groups per call (4 windows -> 128 virtual dests/call)
WPC = CG // W_GROUPS  # windows per call = 4

BF16 = ml_dtypes.bfloat16


# ----------------------------------------------------------------------------
# Host-side packing
# ----------------------------------------------------------------------------

def pack_core(rows_loc, cols, vals, npc):
    """Pack one core's edges (dest-local ids in [0, npc)) into windows.

    Returns per-slot arrays (idx/val/seg, [G*128]) plus the virtual-row ->
    true-dest mapping (vrow/dest).
    """
    deg = np.bincount(rows_loc, minlength=npc).astype(np.int64)
    order = np.argsort(rows_loc, kind="stable")
    cols_s = cols[order]
    vals_s = vals[order]
    starts = np.zeros(npc + 1, np.int64)
    starts[1:] = np.cumsum(deg)

    # items = (dest, edge_start, item_deg); split dests too big for a window
    if deg.max(initial=0) <= W_SLOTS:
        items_dest = np.arange(npc, dtype=np.int64)
        items_start = starts[:-1].copy()
        items_deg = deg.copy()
    else:
        items_dest, items_start, items_deg = [], [], []
        for d in range(npc):
            dd, off = int(deg[d]), 0
            while True:
                take = min(dd - off, W_SLOTS)
                items_dest.append(d)
                items_start.append(starts[d] + off)
                items_deg.append(take)
                off += take
                if off >= dd:
                    break
        items_dest = np.asarray(items_dest, np.int64)
        items_start = np.asarray(items_start, np.int64)
        items_deg = np.asarray(items_deg, np.int64)

    n_items = len(items_dest)
    # two-pointer packing on degree-sorted items: biggest + fill with smallest
    asc = np.argsort(items_deg, kind="stable")
    a, b = 0, n_items - 1
    w_of = np.empty(n_items, np.int64)
    rank_of = np.empty(n_items, np.int64)
    base_of = np.empty(n_items, np.int64)  # slot base within global slot space
    packed_order = np.empty(n_items, np.int64)
    w = 0
    pos = 0
    while a <= b:
        it = asc[b]
        b -= 1
        cap = W_SLOTS - items_deg[it]
        members = [it]
        while a <= b and len(members) < W_RANK and items_deg[asc[a]] <= cap:
            cap -= items_deg[asc[a]]
            members.append(asc[a])
            a += 1
        sb = 0
        for r, it2 in enumerate(members):
            w_of[it2] = w
            rank_of[it2] = r
            base_of[it2] = w * W_SLOTS + sb
            sb += items_deg[it2]
            packed_order[pos] = it2
            pos += 1
        w += 1
    n_windows = w

    # edge-level slot assignment (vectorized over packed items)
    po = packed_order
    ideg = items_deg[po]
    tot = int(ideg.sum())
    reps = np.repeat(np.arange(len(po)), ideg)
    csum = np.zeros(len(po) + 1, np.int64)
    csum[1:] = np.cumsum(ideg)
    within = np.arange(tot, dtype=np.int64) - csum[reps]
    e_pos = items_start[po][reps] + within
    slot = base_of[po][reps] + within

    G = n_windows * W_GROUPS
    idx_slot = np.zeros(G * SLOTS, np.int32)
    val_slot = np.zeros(G * SLOTS, np.float32)
    seg_slot = np.zeros(G * SLOTS, np.int16)
    idx_slot[slot] = cols_s[e_pos]
    val_slot[slot] = vals_s[e_pos]
    seg_slot[slot] = np.repeat(rank_of[po], ideg)

    # virtual output row of item: call*128 + 32*(w % WPC) + rank
    vrow = (w_of // WPC) * 128 + (w_of % WPC) * W_RANK + rank_of
    return dict(
        n_windows=n_windows,
        G=G,
        idx=idx_slot,
        val=val_slot,
        seg=seg_slot,
        vrow=vrow,
        dest=items_dest,
        n_edges=len(rows_loc),
    )


def pack_all(edge_row, edge_col, edge_val, n_nodes=N_NODES, n_cores=N_CORES):
    npc = n_nodes // n_cores
    core_id = edge_row // npc
    packs = []
    for i in range(n_cores):
        m = core_id == i
        packs.append(
            pack_core(edge_row[m] - i * npc, edge_col[m], edge_val[m], npc)
        )
    return packs


def build_call_arrays(p, n_calls):
    """Reshape per-slot arrays into DRAM layouts [n_calls, 128, CG].

    Slot (group g, lane p) of a call maps to gather row j = p*CG + g (the
    indirect DMA fills the [128, CG, 256] tile in partition-major flat
    order), so idx/seg/val all share the [call][p][g] layout.
    """
    G = p["G"]
    gtot = n_calls * CG

    def lay(a, np_dtype):
        full = np.zeros(gtot * SLOTS, a.dtype)
        full[: G * SLOTS] = a
        # [call, g_local, p] -> [call, p, g_local]
        return np.ascontiguousarray(
            full.reshape(n_calls, CG, SLOTS).transpose(0, 2, 1)
        ).astype(np_dtype)

    return (
        lay(p["idx"], np.int32),
        lay(p["seg"], BF16),
        lay(p["val"], BF16),
    )


# ----------------------------------------------------------------------------
# Device program
# ----------------------------------------------------------------------------

def build_program(n_calls, n_nodes=N_NODES, d=D):
    nc = bacc.Bacc("TRN2", target_bir_lowering=False, debug=False)
    f32 = mybir.dt.float32
    bf16 = mybir.dt.bfloat16

    x = nc.dram_tensor("xb", [n_nodes, d], bf16, kind="ExternalInput")
    idxT = nc.dram_tensor("idx", [n_calls, SLOTS, CG], mybir.dt.int32, kind="ExternalInput")
    segT = nc.dram_tensor("seg", [n_calls, SLOTS, CG], bf16, kind="ExternalInput")
    valT = nc.dram_tensor("val", [n_calls, SLOTS, CG], bf16, kind="ExternalInput")
    wtT = nc.dram_tensor("wt", [d // 128, 128, d], bf16, kind="ExternalInput")
    iotaT = nc.dram_tensor("iota32", [128, W_RANK], bf16, kind="ExternalInput")
    identT = nc.dram_tensor("ident", [128, 128], bf16, kind="ExternalInput")
    out = nc.dram_tensor("out", [n_calls * 128, d], f32, kind="ExternalOutput")

    kh = d // 128  # feature half-tiles

    with tile.TileContext(nc) as tc, ExitStack() as ctx:
        const = ctx.enter_context(tc.tile_pool(name="const", bufs=1))
        sb = ctx.enter_context(tc.tile_pool(name="sb", bufs=3))
        xgp = ctx.enter_context(tc.tile_pool(name="xg", bufs=3))
        ps = ctx.enter_context(tc.tile_pool(name="ps", bufs=2, space="PSUM"))

        wt_t = const.tile([128, kh * d], bf16)
        for h in range(kh):
            nc.sync.dma_start(wt_t[:, h * d : (h + 1) * d], wtT[h])
        iota_t = const.tile([128, W_RANK], bf16)
        nc.sync.dma_start(iota_t[:], iotaT[:, :])
        id_t = const.tile([128, 128], bf16)
        nc.sync.dma_start(id_t[:], identT[:, :])

        for c in range(n_calls):
            idx_t = sb.tile([SLOTS, CG], mybir.dt.int32, tag="idx")
            nc.sync.dma_start(idx_t[:], idxT[c])
            seg_t = sb.tile([SLOTS, CG], bf16, tag="seg")
            nc.sync.dma_start(seg_t[:], segT[c])
            val_t = sb.tile([SLOTS, CG], bf16, tag="val")
            nc.sync.dma_start(val_t[:], valT[c])

            xg = xgp.tile([SLOTS, CG, d], bf16, tag="xg")
            # one indirect DMA per group: offsets [128,1] -> one gathered row
            # per partition (the only shape with verified HW semantics)
            for g in range(CG):
                nc.gpsimd.indirect_dma_start(
                    out=xg[:, g, :],
                    out_offset=None,
                    in_=x[:, :],
                    in_offset=bass.IndirectOffsetOnAxis(
                        ap=idx_t[:, g : g + 1], axis=0
                    ),
                )

            # banded scaled one-hot: S[p, g, r] = val[p,g] * (seg[p,g] == r)
            d1 = sb.tile([SLOTS, CG, W_RANK], bf16, tag="d1")
            nc.vector.tensor_tensor(
                out=d1[:],
                in0=seg_t[:].unsqueeze(2).to_broadcast([SLOTS, CG, W_RANK]),
                in1=iota_t[:].unsqueeze(1).to_broadcast([SLOTS, CG, W_RANK]),
                op=mybir.AluOpType.subtract,
            )
            s_t = sb.tile([SLOTS, CG, W_RANK], bf16, tag="s")
            nc.vector.scalar_tensor_tensor(
                out=s_t[:],
                in0=d1[:],
                scalar=0.0,
                op0=mybir.AluOpType.is_equal,
                in1=val_t[:].unsqueeze(2).to_broadcast([SLOTS, CG, W_RANK]),
                op1=mybir.AluOpType.mult,
            )

            # base_partition() only supports 0/32/64, so two 64-partition
            # accumulators (windows 0,1 -> pacc_a; 2,3 -> pacc_b)
            pacc_a = ps.tile([64, d], f32, tag="pacc_a")
            pacc_b = ps.tile([64, d], f32, tag="pacc_b")
            for g in range(CG):
                wloc = g // W_GROUPS  # window within call (0..3)
                j = g % W_GROUPS
                pacc = pacc_a if wloc < 2 else pacc_b
                off = (wloc % 2) * W_RANK
                nc.tensor.matmul(
                    out=pacc[off : off + W_RANK, :],
                    lhsT=s_t[:, g, :],
                    rhs=xg[:, g, :],
                    start=(j == 0),
                    stop=(j == W_GROUPS - 1),
                )

            # cast aggregate to bf16, transpose on PE, multiply by W.T
            t1 = sb.tile([128, d], bf16, tag="t1")
            nc.vector.tensor_copy(out=t1[0:64, :], in_=pacc_a[:])
            nc.vector.tensor_copy(out=t1[64:128, :], in_=pacc_b[:])
            pT = ps.tile([128, kh * 128], bf16, tag="pT")
            for h in range(kh):
                nc.tensor.transpose(
                    out=pT[:, h * 128 : (h + 1) * 128],
                    in_=t1[:, h * 128 : (h + 1) * 128],
                    identity=id_t[:],
                )
            aggT = sb.tile([128, kh * 128], bf16, tag="aggT")
            nc.vector.tensor_copy(out=aggT[:], in_=pT[:])

            pout = ps.tile([128, d], f32, tag="pout")
            for h in range(kh):
                nc.tensor.matmul(
                    out=pout[:],
                    lhsT=aggT[:, h * 128 : (h + 1) * 128],
                    rhs=wt_t[:, h * d : (h + 1) * d],
                    start=(h == 0),
                    stop=(h == kh - 1),
                )
            osb = sb.tile([128, d], f32, tag="osb")
            nc.vector.tensor_copy(out=osb[:], in_=pout[:])
            nc.scalar.dma_start(out[c * 128 : (c + 1) * 128, :], osb[:])

    nc.compile()
    return nc


# ----------------------------------------------------------------------------
# Entry point
# ----------------------------------------------------------------------------

_PROG_CACHE = {}


def _get_program(n_calls):
    if n_calls not in _PROG_CACHE:
        _PROG_CACHE[n_calls] = build_program(n_calls)
    return _PROG_CACHE[n_calls]


def make_in_maps(x, W, packs, n_calls):
    xb = np.ascontiguousarray(x.astype(BF16))
    wt = np.ascontiguousarray(W.T.reshape(D // 128, 128, D).astype(BF16))
    iota = np.broadcast_to(np.arange(W_RANK, dtype=np.float32), (128, W_RANK))
    iota = np.ascontiguousarray(iota.astype(BF16))
    ident = np.eye(128, dtype=np.float32).astype(BF16)
    in_maps = []
    for p in packs:
        idx, seg, val = build_call_arrays(p, n_calls)
        in_maps.append(
            dict(xb=xb, idx=idx, seg=seg, val=val, wt=wt, iota32=iota, ident=ident)
        )
    return in_maps


def kernel(x, W, edge_val, edge_row, edge_col, _return_results=False, trace=False):
    packs = pack_all(edge_row, edge_col, edge_val)
    n_windows_max = max(p["n_windows"] for p in packs)
    n_calls = (n_windows_max + WPC - 1) // WPC
    nc = _get_program(n_calls)
    in_maps = make_in_maps(x, W, packs, n_calls)
    res = run_bass_kernel_spmd(
        nc, in_maps, core_ids=list(range(N_CORES)), trace=trace
    )
    out = np.zeros((N_NODES, D), np.float32)
    for i, (p, core_out) in enumerate(zip(packs, res.results)):
        ov = core_out["out"]
        true_ids = p["dest"] + i * NPC
        if len(np.unique(true_ids)) == len(true_ids):
            out[true_ids] = ov[p["vrow"]]
        else:
            np.add.at(out, true_ids, ov[p["vrow"]])
    if _return_results:
        return out, res
    return out


# revision 9
# speedup vs baseline: 13041.6191x; 13041.6191x over previous
"""GCN layer (linear + weighted scatter-add aggregation) on 8 TRN2 NeuronCores.

Reference computation:
    h = x @ W.T                      [N, D]
    out[r] = sum_{e: row[e]==r} val[e] * h[col[e]]

Key identity: the linear layer commutes past the (linear) aggregation:
    out = (A @ x) @ W.T    where A[r,c] = sum of val over edges (r,c)
so we aggregate raw x rows first (8x less matmul work, no h materialization).

Distribution: destination nodes are sharded 12500/core (edges partitioned by
destination so the segment-sum is fully local; x is replicated to each core's
HBM by the host, so no collective is needed).

Per-core algorithm ("perm-pack"):
  - Host packs *whole* destinations into "windows" of <=32 dests and 4x256
    edge slots, where the 4 quotas correspond to 4 source-node chunks of
    25000 rows (dma_gather indices are int16). The packing order defines a
    per-core virtual destination numbering; output rows are inverse-permuted
    (and summed, if a dest was split) on the host.
  - A window is 8 "groups" of 128 edge slots (2 per chunk). One "call" = 4
    windows = 32 groups = 4096 slots: four batched GPSIMD dma_gather ucode
    calls (1024 int16 indices each, one per source chunk) pull the x rows
    (bf16, 512B each) into SBUF as [128 slots, 32 groups, 256]. A banded
    scaled one-hot S ([128, 32] per group: S[p,r] = val[p] * (rank[p]==r))
    is built with 2 batched DVE ops per call.
  - PE: per group one matmul (lhsT = S band, rhs = gathered rows
    [128,256]) accumulating over the window's 8 groups into a 32-partition
    PSUM region; 4 windows fill 128 virtual dests' aggregate per call. The
    aggregate is cast to bf16, transposed on PE (identity trick), and
    multiplied by W.T (bf16, f32 PSUM) into the output block. No scatter,
    no atomics, no collectives.
"""

import os
import sys

sys.path.insert(0, "/opt/trn_rl_repo")
os.environ.setdefault("MYCRO_LOCAL_CACHE", "1")

from contextlib import ExitStack

import numpy as np
import ml_dtypes

import concourse.bass as bass
import concourse.bacc as bacc
import concourse.mybir as mybir
import concourse.tile as tile
from concourse.bass_utils import run_bass_kernel_spmd
from concourse.library_config import mlp as _mlp_lib

N_NODES = 100000
N_CORES = 8
NPC = N_NODES // N_CORES  # dests per core
D = 256
SLOTS = 128  # edge slots per group (= matmul K)
W_RANK = 32  # dests per window (= matmul M)
NCHUNK = 4
CHUNK = 25000  # source rows per chunk (int16-addressable)
W_CQ = 256  # window chunk quota (2 groups per chunk)
W_GROUPS = 8  # groups per window
W_SLOTS = W_GROUPS * SLOTS  # 1024 edge slots per window
CG = 32  # groups per call (4 windows -> 128 virtual dests/call)
WPC = 4  # windows per call
GATHER_IDX = WPC * W_CQ  # 1024 indices per (call, chunk) dma_gather

BF16 = ml_dtypes.bfloat16


# ----------------------------------------------------------------------------
# Host-side packing
# ----------------------------------------------------------------------------

def pack_core(rows_loc, cols, vals, npc):
    """Pack one core's edges (dest-local ids in [0, npc)) into windows.

    Returns per-slot arrays (idx/val/seg, [G*128]) in (call, chunk, window)
    slot order, plus the virtual-row -> true-dest mapping (vrow/dest).
    """
    chunk_id = cols // CHUNK
    key = rows_loc.astype(np.int64) * NCHUNK + chunk_id
    order = np.argsort(key, kind="stable")
    cols_s = cols[order]
    vals_s = vals[order]
    dc_deg = np.bincount(key, minlength=npc * NCHUNK).astype(np.int64)
    dc_deg = dc_deg.reshape(npc, NCHUNK)
    dc_start = np.zeros(npc * NCHUNK + 1, np.int64)
    dc_start[1:] = np.cumsum(dc_deg.ravel())
    dc_start = dc_start[:-1].reshape(npc, NCHUNK)
    deg = dc_deg.sum(1)

    if dc_deg.max(initial=0) > W_CQ:
        raise RuntimeError("single dest exceeds per-chunk window quota")

    # two-pointer packing on total-degree-sorted dests; a dest fits a window
    # iff every per-chunk sum stays <= W_CQ
    asc = np.argsort(deg, kind="stable")
    a, b = 0, npc - 1
    w_of = np.empty(npc, np.int64)
    rank_of = np.empty(npc, np.int64)
    windows = []  # list of member-lists
    while a <= b:
        it = int(asc[b])
        b -= 1
        members = [it]
        cq = dc_deg[it].copy()
        while a <= b and len(members) < W_RANK:
            cand = int(asc[a])
            if np.all(cq + dc_deg[cand] <= W_CQ):
                cq += dc_deg[cand]
                members.append(cand)
                a += 1
            else:
                break
        w = len(windows)
        for r, it2 in enumerate(members):
            w_of[it2] = w
            rank_of[it2] = r
        windows.append(members)
    n_windows = len(windows)

    # per-(dest, chunk) slot base in the (call, chunk, window) slot order:
    # call*4096 + (8c + 2*w_loc)*128 + running-offset-within-window-chunk
    item_qbase = np.empty((npc, NCHUNK), np.int64)
    for w, members in enumerate(windows):
        call, w_loc = w // WPC, w % WPC
        base = call * (CG * SLOTS)
        cum = np.zeros(NCHUNK, np.int64)
        for it2 in members:
            for c in range(NCHUNK):
                item_qbase[it2, c] = base + (8 * c + 2 * w_loc) * SLOTS + cum[c]
            cum += dc_deg[it2]

    # edge-level slot assignment, vectorized over (dest, chunk) cells
    flat_deg = dc_deg.ravel()
    flat_start = dc_start.ravel()
    flat_qbase = item_qbase.ravel()
    nz = np.nonzero(flat_deg)[0]
    nz_deg = flat_deg[nz]
    reps = np.repeat(np.arange(len(nz)), nz_deg)
    csum = np.zeros(len(nz) + 1, np.int64)
    csum[1:] = np.cumsum(nz_deg)
    within = np.arange(int(nz_deg.sum()), dtype=np.int64) - csum[reps]
    e_pos = flat_start[nz][reps] + within
    slot = flat_qbase[nz][reps] + within

    n_calls_local = (n_windows + WPC - 1) // WPC
    G = n_calls_local * CG
    idx_slot = np.zeros(G * SLOTS, np.int32)
    val_slot = np.zeros(G * SLOTS, np.float32)
    seg_slot = np.zeros(G * SLOTS, np.int16)
    idx_slot[slot] = cols_s[e_pos] % CHUNK
    val_slot[slot] = vals_s[e_pos]
    seg_slot[slot] = rank_of[nz[reps] // NCHUNK]

    vrow = (w_of // WPC) * 128 + (w_of % WPC) * W_RANK + rank_of
    return dict(
        n_windows=n_windows,
        G=G,
        idx=idx_slot,
        val=val_slot,
        seg=seg_slot,
        vrow=vrow,
        dest=np.arange(npc, dtype=np.int64),
        n_edges=len(rows_loc),
    )


def pack_all(edge_row, edge_col, edge_val, n_nodes=N_NODES, n_cores=N_CORES):
    npc = n_nodes // n_cores
    core_id = edge_row // npc
    packs = []
    for i in range(n_cores):
        m = core_id == i
        packs.append(
            pack_core(edge_row[m] - i * npc, edge_col[m], edge_val[m], npc)
        )
    return packs


def build_call_arrays(p, n_calls):
    """DRAM layouts: seg/val [n_calls, 128, CG]; idx wrapped int16
    [n_calls, NCHUNK, 128, GATHER_IDX//16].

    Slot s of group g lives at flat position (call*CG + g)*128 + s; the
    dma_gather for (call, chunk c) consumes groups 8c..8c+8 in order, index
    position i -> (partition i%128, group 8c + i//128), wrapped so that
    position i sits at [i%16, i//16] (replicated over each 16-partition
    block).
    """
    G = p["G"]
    gtot = n_calls * CG

    def lay(a, np_dtype):
        full = np.zeros(gtot * SLOTS, a.dtype)
        full[: G * SLOTS] = a
        return np.ascontiguousarray(
            full.reshape(n_calls, CG, SLOTS).transpose(0, 2, 1)
        ).astype(np_dtype)

    idx_full = np.zeros(gtot * SLOTS, np.int64)
    idx_full[: G * SLOTS] = p["idx"]
    byg = idx_full.reshape(n_calls, CG, SLOTS)
    idx = np.empty((n_calls, NCHUNK, 128, GATHER_IDX // 16), np.int16)
    for c in range(NCHUNK):
        flat = byg[:, 8 * c : 8 * (c + 1), :].reshape(n_calls, GATHER_IDX)
        wrapped = flat.reshape(n_calls, GATHER_IDX // 16, 16).transpose(0, 2, 1)
        idx[:, c, :, :] = np.tile(wrapped, (1, 8, 1)).astype(np.int16)

    return (
        np.ascontiguousarray(idx),
        lay(p["seg"], BF16),
        lay(p["val"], BF16),
    )


# ----------------------------------------------------------------------------
# Device program
# ----------------------------------------------------------------------------

def build_program(n_calls, n_nodes=N_NODES, d=D, chunk=CHUNK):
    nc = bacc.Bacc("TRN2", target_bir_lowering=False, debug=False)
    f32 = mybir.dt.float32
    bf16 = mybir.dt.bfloat16

    x = nc.dram_tensor("xb", [n_nodes, d], bf16, kind="ExternalInput")
    idxT = nc.dram_tensor(
        "idx",
        [n_calls, NCHUNK, 128, GATHER_IDX // 16],
        mybir.dt.int16,
        kind="ExternalInput",
    )
    segT = nc.dram_tensor("seg", [n_calls, SLOTS, CG], bf16, kind="ExternalInput")
    valT = nc.dram_tensor("val", [n_calls, SLOTS, CG], bf16, kind="ExternalInput")
    wtT = nc.dram_tensor("wt", [d // 128, 128, d], bf16, kind="ExternalInput")
    iotaT = nc.dram_tensor("iota32", [128, W_RANK], bf16, kind="ExternalInput")
    identT = nc.dram_tensor("ident", [128, 128], bf16, kind="ExternalInput")
    out = nc.dram_tensor("out", [n_calls * 128, d], f32, kind="ExternalOutput")

    kh = d // 128  # feature half-tiles
    n_chunks = (n_nodes + chunk - 1) // chunk
    iw = GATHER_IDX // 16  # idx words per chunk-call per partition

    with tile.TileContext(nc) as tc, ExitStack() as ctx:
        const = ctx.enter_context(tc.tile_pool(name="const", bufs=1))
        sb = ctx.enter_context(tc.tile_pool(name="sb", bufs=3))
        xgp = ctx.enter_context(tc.tile_pool(name="xg", bufs=3))
        ps = ctx.enter_context(tc.tile_pool(name="ps", bufs=2, space="PSUM"))

        nc.gpsimd.load_library(_mlp_lib)

        wt_t = const.tile([128, kh * d], bf16)
        for h in range(kh):
            nc.sync.dma_start(wt_t[:, h * d : (h + 1) * d], wtT[h])
        iota_t = const.tile([128, W_RANK], bf16)
        nc.sync.dma_start(iota_t[:], iotaT[:, :])
        id_t = const.tile([128, 128], bf16)
        nc.sync.dma_start(id_t[:], identT[:, :])

        for cl in range(n_calls):
            idx_t = sb.tile([128, NCHUNK * iw], mybir.dt.int16, tag="idx")
            for c in range(NCHUNK):
                nc.sync.dma_start(idx_t[:, c * iw : (c + 1) * iw], idxT[cl, c])
            seg_t = sb.tile([SLOTS, CG], bf16, tag="seg")
            nc.sync.dma_start(seg_t[:], segT[cl])
            val_t = sb.tile([SLOTS, CG], bf16, tag="val")
            nc.sync.dma_start(val_t[:], valT[cl])

            xg = xgp.tile([SLOTS, CG, d], bf16, tag="xg")
            for c in range(min(n_chunks, NCHUNK)):
                lo = c * chunk
                hi = min(n_nodes, lo + chunk)
                nc.gpsimd.dma_gather(
                    xg[:, 8 * c : 8 * (c + 1), :],
                    x[lo:hi, :],
                    idx_t[:, c * iw : (c + 1) * iw],
                    GATHER_IDX,
                    GATHER_IDX,
                    d,
                )

            # banded scaled one-hot: S[p, g, r] = val[p,g] * (seg[p,g] == r)
            d1 = sb.tile([SLOTS, CG, W_RANK], bf16, tag="d1")
            nc.vector.tensor_tensor(
                out=d1[:],
                in0=seg_t[:].unsqueeze(2).to_broadcast([SLOTS, CG, W_RANK]),
                in1=iota_t[:].unsqueeze(1).to_broadcast([SLOTS, CG, W_RANK]),
                op=mybir.AluOpType.subtract,
            )
            s_t = sb.tile([SLOTS, CG, W_RANK], bf16, tag="s")
            nc.vector.scalar_tensor_tensor(
                out=s_t[:],
                in0=d1[:],
                scalar=0.0,
                op0=mybir.AluOpType.is_equal,
                in1=val_t[:].unsqueeze(2).to_broadcast([SLOTS, CG, W_RANK]),
                op1=mybir.AluOpType.mult,
            )

            # base_partition() only supports 0/32/64, so two 64-partition
            # accumulators (windows 0,1 -> pacc_a; 2,3 -> pacc_b)
            pacc_a = ps.tile([64, d], f32, tag="pacc_a")
            pacc_b = ps.tile([64, d], f32, tag="pacc_b")
            for w_loc in range(WPC):
                pacc = pacc_a if w_loc < 2 else pacc_b
                off = (w_loc % 2) * W_RANK
                for c in range(NCHUNK):
                    for j in range(2):
                        g = 8 * c + 2 * w_loc + j
                        nc.tensor.matmul(
                            out=pacc[off : off + W_RANK, :],
                            lhsT=s_t[:, g, :],
                            rhs=xg[:, g, :],
                            start=(c == 0 and j == 0),
                            stop=(c == NCHUNK - 1 and j == 1),
                        )

            # cast aggregate to bf16, transpose on PE, multiply by W.T
            t1 = sb.tile([128, d], bf16, tag="t1")
            nc.vector.tensor_copy(out=t1[0:64, :], in_=pacc_a[:])
            nc.vector.tensor_copy(out=t1[64:128, :], in_=pacc_b[:])
            pT = ps.tile([128, kh * 128], bf16, tag="pT")
            for h in range(kh):
                nc.tensor.transpose(
                    out=pT[:, h * 128 : (h + 1) * 128],
                    in_=t1[:, h * 128 : (h + 1) * 128],
                    identity=id_t[:],
                )
            aggT = sb.tile([128, kh * 128], bf16, tag="aggT")
            nc.vector.tensor_copy(out=aggT[:], in_=pT[:])

            pout = ps.tile([128, d], f32, tag="pout")
            for h in range(kh):
                nc.tensor.matmul(
                    out=pout[:],
                    lhsT=aggT[:, h * 128 : (h + 1) * 128],
                    rhs=wt_t[:, h * d : (h + 1) * d],
                    start=(h == 0),
                    stop=(h == kh - 1),
                )
            osb = sb.tile([128, d], f32, tag="osb")
            nc.vector.tensor_copy(out=osb[:], in_=pout[:])
            nc.scalar.dma_start(out[cl * 128 : (cl + 1) * 128, :], osb[:])

    nc.compile()
    return nc


# ----------------------------------------------------------------------------
# Entry point
# ----------------------------------------------------------------------------

_PROG_CACHE = {}


def _get_program(n_calls):
    if n_calls not in _PROG_CACHE:
        _PROG_CACHE[n_calls] = build_program(n_calls)
    return _PROG_CACHE[n_calls]


def make_in_maps(x, W, packs, n_calls):
    xb = np.ascontiguousarray(x.astype(BF16))
    wt = np.ascontiguousarray(W.T.reshape(D // 128, 128, D).astype(BF16))
    iota = np.broadcast_to(np.arange(W_RANK, dtype=np.float32), (128, W_RANK))
    iota = np.ascontiguousarray(iota.astype(BF16))
    ident = np.eye(128, dtype=np.float32).astype(BF16)
    in_maps = []
    for p in packs:
        idx, seg, val = build_call_arrays(p, n_calls)
        in_maps.append(
            dict(xb=xb, idx=idx, seg=seg, val=val, wt=wt, iota32=iota, ident=ident)
        )
    return in_maps


def kernel(x, W, edge_val, edge_row, edge_col, _return_results=False, trace=False):
    packs = pack_all(edge_row, edge_col, edge_val)
    n_windows_max = max(p["n_windows"] for p in packs)
    n_calls = (n_windows_max + WPC - 1) // WPC
    nc = _get_program(n_calls)
    in_maps = make_in_maps(x, W, packs, n_calls)
    res = run_bass_kernel_spmd(
        nc, in_maps, core_ids=list(range(N_CORES)), trace=trace
    )
    out = np.zeros((N_NODES, D), np.float32)
    for i, (p, core_out) in enumerate(zip(packs, res.results)):
        ov = core_out["out"]
        true_ids = p["dest"] + i * NPC
        if len(np.unique(true_ids)) == len(true_ids):
            out[true_ids] = ov[p["vrow"]]
        else:
            np.add.at(out, true_ids, ov[p["vrow"]])
    if _return_results:
        return out, res
    return out


# revision 12
# speedup vs baseline: 13820.1499x; 1.0597x over previous
"""GCN layer (linear + weighted scatter-add aggregation) on 8 TRN2 NeuronCores.

Reference computation:
    h = x @ W.T                      [N, D]
    out[r] = sum_{e: row[e]==r} val[e] * h[col[e]]

Key identity: the linear layer commutes past the (linear) aggregation:
    out = (A @ x) @ W.T    where A[r,c] = sum of val over edges (r,c)
so we aggregate raw x rows first (8x less matmul work, no h materialization).

Distribution: destination nodes are sharded 12500/core (edges partitioned by
destination so the segment-sum is fully local; x is replicated to each core's
HBM by the host, so no collective is needed).

Per-core algorithm ("perm-pack"):
  - Host packs *whole* destinations into "windows" of <=32 dests and 4x256
    edge slots, where the 4 quotas correspond to 4 source-node chunks of
    25000 rows (dma_gather indices are int16). The packing order defines a
    per-core virtual destination numbering; output rows are inverse-permuted
    (and summed, if a dest was split) on the host.
  - A window is 8 "groups" of 128 edge slots (2 per chunk). One "call" = 4
    windows = 32 groups = 4096 slots: four batched GPSIMD dma_gather ucode
    calls (1024 int16 indices each, one per source chunk) pull the x rows
    (bf16, 512B each) into SBUF as [128 slots, 32 groups, 256]. A banded
    scaled one-hot S ([128, 32] per group: S[p,r] = val[p] * (rank[p]==r))
    is built with 2 batched DVE ops per call.
  - PE: per group one matmul (lhsT = S band, rhs = gathered rows
    [128,256]) accumulating over the window's 8 groups into a 32-partition
    PSUM region; 4 windows fill 128 virtual dests' aggregate per call. The
    aggregate is cast to bf16, transposed on PE (identity trick), and
    multiplied by W.T (bf16, f32 PSUM) into the output block. No scatter,
    no atomics, no collectives.
"""

import os
import sys

sys.path.insert(0, "/opt/trn_rl_repo")
os.environ.setdefault("MYCRO_LOCAL_CACHE", "1")

from contextlib import ExitStack

import numpy as np
import ml_dtypes

import concourse.bass as bass
import concourse.bacc as bacc
import concourse.mybir as mybir
import concourse.tile as tile
from concourse.bass_utils import run_bass_kernel_spmd
from concourse.library_config import mlp as _mlp_lib

N_NODES = 100000
N_CORES = 8
NPC = N_NODES // N_CORES  # dests per core
D = 256
SLOTS = 128  # edge slots per group (= matmul K)
W_RANK = 32  # dests per window (= matmul M)
NCHUNK = 4
CHUNK = 25000  # source rows per chunk (int16-addressable)
W_CQ = 256  # window chunk quota (2 groups per chunk)
W_GROUPS = 8  # groups per window
W_SLOTS = W_GROUPS * SLOTS  # 1024 edge slots per window
CG = 32  # groups per call (4 windows -> 128 virtual dests/call)
WPC = 4  # windows per call
F_CALLS = 1  # calls fused per gather instruction (ring caps gathers at 1024 descriptors)
GATHER_IDX = F_CALLS * WPC * W_CQ  # 2048 indices per (pair, chunk) dma_gather

BF16 = ml_dtypes.bfloat16


# ----------------------------------------------------------------------------
# Host-side packing
# ----------------------------------------------------------------------------

def pack_core(rows_loc, cols, vals, npc):
    """Pack one core's edges (dest-local ids in [0, npc)) into windows.

    Dests are placed greedily (alternating big/small by total degree) and may
    be SPLIT across consecutive windows when a per-chunk quota or the rank
    cap is hit, so windows fill to ~100%. Split partial sums are re-combined
    on the host (np.add.at over duplicate dest ids).

    Returns per-slot arrays idx (in F2-fused gather slot order) and seg/val
    (in per-call slot order), plus per-item vrow/dest.
    """
    chunk_id = cols // CHUNK
    key = rows_loc.astype(np.int64) * NCHUNK + chunk_id
    order = np.argsort(key, kind="stable")
    cols_s = cols[order]
    vals_s = vals[order]
    dc_deg = np.bincount(key, minlength=npc * NCHUNK).astype(np.int64)
    dc_deg = dc_deg.reshape(npc, NCHUNK)
    dc_start = np.zeros(npc * NCHUNK + 1, np.int64)
    dc_start[1:] = np.cumsum(dc_deg.ravel())
    dc_start = dc_start[:-1].reshape(npc, NCHUNK)
    deg = dc_deg.sum(1)

    # alternating big/small feed order balances window sums
    srt = np.argsort(deg, kind="stable")
    feed = np.empty(npc, np.int64)
    feed[0::2] = srt[::-1][: (npc + 1) // 2]
    feed[1::2] = srt[: npc // 2]

    # greedy fill with splitting
    items_dest, items_w, items_rank = [], [], []
    items_take = []  # [NCHUNK] takes
    items_coff = []  # [NCHUNK] consumed offset within dest-chunk edges
    w = 0
    rank = 0
    rq = [W_CQ] * NCHUNK
    for d in feed:
        d = int(d)
        rem = dc_deg[d].copy()
        coff = np.zeros(NCHUNK, np.int64)
        while True:
            if rank == W_RANK:
                w += 1
                rank = 0
                rq = [W_CQ] * NCHUNK
            take = np.minimum(rem, rq)
            items_dest.append(d)
            items_w.append(w)
            items_rank.append(rank)
            items_take.append(take.copy())
            items_coff.append(coff.copy())
            rq = [int(rq[c] - take[c]) for c in range(NCHUNK)]
            rank += 1
            rem -= take
            coff += take
            if rem.sum() == 0:
                break
            # some chunk quota exhausted: close window, continue this dest
            w += 1
            rank = 0
            rq = [W_CQ] * NCHUNK
    n_windows = w + 1
    n_items = len(items_dest)
    items_dest = np.asarray(items_dest, np.int64)
    items_w = np.asarray(items_w, np.int64)
    items_rank = np.asarray(items_rank, np.int64)
    items_take = np.asarray(items_take, np.int64)  # [n_items, NCHUNK]
    items_coff = np.asarray(items_coff, np.int64)

    n_calls_local = (n_windows + WPC - 1) // WPC
    n_pairs = (n_calls_local + F_CALLS - 1) // F_CALLS
    n_calls_local = n_pairs * F_CALLS
    G = n_calls_local * CG

    # per-(item, chunk) slot bases in both slot orders
    call = items_w // WPC
    w_loc = items_w % WPC
    pair = call // F_CALLS
    q = call % F_CALLS
    # running offset within each window chunk quota
    qoff = np.zeros((n_items, NCHUNK), np.int64)
    cum = {}
    for i in range(n_items):
        ww = items_w[i]
        c0 = cum.get(ww)
        if c0 is None:
            c0 = np.zeros(NCHUNK, np.int64)
        qoff[i] = c0
        cum[ww] = c0 + items_take[i]
    # seg/val order: call*4096 + (8c + 2*w_loc)*128 + qoff
    base_sv = call * (CG * SLOTS)
    # idx order: pair*F*4096 + (8F*c + 8*q + 2*w_loc)*128 + qoff
    base_ix = pair * (F_CALLS * CG * SLOTS)

    flat_deg = items_take.ravel()
    cgrid = np.tile(np.arange(NCHUNK), n_items)
    irep = np.repeat(np.arange(n_items), NCHUNK)
    e_start = (dc_start[items_dest] + items_coff).ravel()
    sv_base = (
        base_sv[irep]
        + (8 * cgrid + 2 * w_loc[irep]) * SLOTS
        + qoff.ravel()
    )
    ix_base = (
        base_ix[irep]
        + (8 * F_CALLS * cgrid + 8 * q[irep] + 2 * w_loc[irep]) * SLOTS
        + qoff.ravel()
    )
    nz = np.nonzero(flat_deg)[0]
    nz_deg = flat_deg[nz]
    reps = np.repeat(np.arange(len(nz)), nz_deg)
    csum = np.zeros(len(nz) + 1, np.int64)
    csum[1:] = np.cumsum(nz_deg)
    within = np.arange(int(nz_deg.sum()), dtype=np.int64) - csum[reps]
    e_pos = e_start[nz][reps] + within
    slot_sv = sv_base[nz][reps] + within
    slot_ix = ix_base[nz][reps] + within

    idx_slot = np.zeros(G * SLOTS, np.int32)
    val_slot = np.zeros(G * SLOTS, np.float32)
    seg_slot = np.zeros(G * SLOTS, np.int16)
    idx_slot[slot_ix] = cols_s[e_pos] % CHUNK
    val_slot[slot_sv] = vals_s[e_pos]
    seg_slot[slot_sv] = items_rank[irep[nz]][reps]

    vrow = (items_w // WPC) * 128 + (items_w % WPC) * W_RANK + items_rank
    return dict(
        n_windows=n_windows,
        G=G,
        idx=idx_slot,
        val=val_slot,
        seg=seg_slot,
        vrow=vrow,
        dest=items_dest,
        n_edges=len(rows_loc),
    )


def pack_all(edge_row, edge_col, edge_val, n_nodes=N_NODES, n_cores=N_CORES):
    npc = n_nodes // n_cores
    core_id = edge_row // npc
    packs = []
    for i in range(n_cores):
        m = core_id == i
        packs.append(
            pack_core(edge_row[m] - i * npc, edge_col[m], edge_val[m], npc)
        )
    return packs


def build_call_arrays(p, n_calls):
    """DRAM layouts: seg/val [n_calls, 128, CG] (per-call slot order); idx
    wrapped int16 [n_pairs, NCHUNK, 128, GATHER_IDX//16] (pair-fused order).

    The dma_gather for (pair, chunk c) consumes the pair's 16 chunk-c groups
    in order; index position i -> (partition i%128, group 16c + i//128 of the
    pair tile), wrapped so position i sits at [i%16, i//16] (replicated over
    each 16-partition block).
    """
    G = p["G"]
    gtot = n_calls * CG
    n_pairs = n_calls // F_CALLS

    def lay(a, np_dtype):
        full = np.zeros(gtot * SLOTS, a.dtype)
        full[: G * SLOTS] = a
        return np.ascontiguousarray(
            full.reshape(n_calls, CG, SLOTS).transpose(0, 2, 1)
        ).astype(np_dtype)

    idx_full = np.zeros(gtot * SLOTS, np.int64)
    idx_full[: G * SLOTS] = p["idx"]
    byg = idx_full.reshape(n_pairs, F_CALLS * CG, SLOTS)
    iw = GATHER_IDX // 16
    idx = np.empty((n_pairs, NCHUNK, 128, iw), np.int16)
    gpc = 4 * F_CALLS * 2  # groups per chunk per pair (16)
    for c in range(NCHUNK):
        flat = byg[:, gpc * c : gpc * (c + 1), :].reshape(n_pairs, GATHER_IDX)
        wrapped = flat.reshape(n_pairs, iw, 16).transpose(0, 2, 1)
        idx[:, c, :, :] = np.tile(wrapped, (1, 8, 1)).astype(np.int16)

    return (
        np.ascontiguousarray(idx),
        lay(p["seg"], BF16),
        lay(p["val"], BF16),
    )


# ----------------------------------------------------------------------------
# Device program
# ----------------------------------------------------------------------------

def build_program(n_calls, n_nodes=N_NODES, d=D, chunk=CHUNK):
    nc = bacc.Bacc("TRN2", target_bir_lowering=False, debug=False)
    f32 = mybir.dt.float32
    bf16 = mybir.dt.bfloat16

    x = nc.dram_tensor("xb", [n_nodes, d], bf16, kind="ExternalInput")
    n_pairs = n_calls // F_CALLS
    iw = GATHER_IDX // 16  # idx words per chunk-gather per partition
    idxT = nc.dram_tensor(
        "idx", [n_pairs, NCHUNK, 128, iw], mybir.dt.int16, kind="ExternalInput"
    )
    segT = nc.dram_tensor("seg", [n_calls, SLOTS, CG], bf16, kind="ExternalInput")
    valT = nc.dram_tensor("val", [n_calls, SLOTS, CG], bf16, kind="ExternalInput")
    wtT = nc.dram_tensor("wt", [d // 128, 128, d], bf16, kind="ExternalInput")
    iotaT = nc.dram_tensor("iota32", [128, W_RANK], bf16, kind="ExternalInput")
    identT = nc.dram_tensor("ident", [128, 128], bf16, kind="ExternalInput")
    out = nc.dram_tensor("out", [n_calls * 128, d], f32, kind="ExternalOutput")

    kh = d // 128  # feature half-tiles
    n_chunks = (n_nodes + chunk - 1) // chunk
    fcg = F_CALLS * CG  # groups per pair tile
    gpc = fcg // NCHUNK  # groups per chunk within a pair tile (16)

    with tile.TileContext(nc) as tc, ExitStack() as ctx:
        const = ctx.enter_context(tc.tile_pool(name="const", bufs=1))
        sb = ctx.enter_context(tc.tile_pool(name="sb", bufs=3))
        xgp = ctx.enter_context(tc.tile_pool(name="xg", bufs=2))
        ps = ctx.enter_context(tc.tile_pool(name="ps", bufs=2, space="PSUM"))

        nc.gpsimd.load_library(_mlp_lib)

        wt_t = const.tile([128, kh * d], bf16)
        for h in range(kh):
            nc.sync.dma_start(wt_t[:, h * d : (h + 1) * d], wtT[h])
        iota_t = const.tile([128, W_RANK], bf16)
        nc.sync.dma_start(iota_t[:], iotaT[:, :])
        id_t = const.tile([128, 128], bf16)
        nc.sync.dma_start(id_t[:], identT[:, :])

        for pr in range(n_pairs):
            idx_t = sb.tile([128, NCHUNK * iw], mybir.dt.int16, tag="idx")
            for c in range(NCHUNK):
                nc.sync.dma_start(idx_t[:, c * iw : (c + 1) * iw], idxT[pr, c])

            xg = xgp.tile([SLOTS, fcg, d], bf16, tag="xg")
            for c in range(min(n_chunks, NCHUNK)):
                lo = c * chunk
                hi = min(n_nodes, lo + chunk)
                nc.gpsimd.dma_gather(
                    xg[:, gpc * c : gpc * (c + 1), :],
                    x[lo:hi, :],
                    idx_t[:, c * iw : (c + 1) * iw],
                    GATHER_IDX,
                    GATHER_IDX,
                    d,
                )

            for q in range(F_CALLS):
                cl = pr * F_CALLS + q
                seg_t = sb.tile([SLOTS, CG], bf16, tag="seg")
                nc.sync.dma_start(seg_t[:], segT[cl])
                val_t = sb.tile([SLOTS, CG], bf16, tag="val")
                nc.sync.dma_start(val_t[:], valT[cl])

                # banded scaled one-hot: S[p, g, r] = val[p,g] * (seg[p,g] == r)
                d1 = sb.tile([SLOTS, CG, W_RANK], bf16, tag="d1")
                nc.vector.tensor_tensor(
                    out=d1[:],
                    in0=seg_t[:].unsqueeze(2).to_broadcast([SLOTS, CG, W_RANK]),
                    in1=iota_t[:].unsqueeze(1).to_broadcast([SLOTS, CG, W_RANK]),
                    op=mybir.AluOpType.subtract,
                )
                s_t = sb.tile([SLOTS, CG, W_RANK], bf16, tag="s")
                nc.vector.scalar_tensor_tensor(
                    out=s_t[:],
                    in0=d1[:],
                    scalar=0.0,
                    op0=mybir.AluOpType.is_equal,
                    in1=val_t[:].unsqueeze(2).to_broadcast([SLOTS, CG, W_RANK]),
                    op1=mybir.AluOpType.mult,
                )

                # base_partition() only supports 0/32/64, so two 64-partition
                # accumulators (windows 0,1 -> pacc_a; 2,3 -> pacc_b)
                pacc_a = ps.tile([64, d], f32, tag="pacc_a")
                pacc_b = ps.tile([64, d], f32, tag="pacc_b")
                for w_loc in range(WPC):
                    pacc = pacc_a if w_loc < 2 else pacc_b
                    off = (w_loc % 2) * W_RANK
                    for c in range(NCHUNK):
                        for j in range(2):
                            gq = 8 * c + 2 * w_loc + j  # per-call group (seg/val)
                            gt = gpc * c + 8 * q + 2 * w_loc + j  # pair-tile group
                            nc.tensor.matmul(
                                out=pacc[off : off + W_RANK, :],
                                lhsT=s_t[:, gq, :],
                                rhs=xg[:, gt, :],
                                start=(c == 0 and j == 0),
                                stop=(c == NCHUNK - 1 and j == 1),
                            )

                # cast aggregate to bf16, transpose on PE, multiply by W.T
                t1 = sb.tile([128, d], bf16, tag="t1")
                nc.vector.tensor_copy(out=t1[0:64, :], in_=pacc_a[:])
                nc.vector.tensor_copy(out=t1[64:128, :], in_=pacc_b[:])
                pT = ps.tile([128, kh * 128], bf16, tag="pT")
                for h in range(kh):
                    nc.tensor.transpose(
                        out=pT[:, h * 128 : (h + 1) * 128],
                        in_=t1[:, h * 128 : (h + 1) * 128],
                        identity=id_t[:],
                    )
                aggT = sb.tile([128, kh * 128], bf16, tag="aggT")
                nc.vector.tensor_copy(out=aggT[:], in_=pT[:])

                pout = ps.tile([128, d], f32, tag="pout")
                for h in range(kh):
                    nc.tensor.matmul(
                        out=pout[:],
                        lhsT=aggT[:, h * 128 : (h + 1) * 128],
                        rhs=wt_t[:, h * d : (h + 1) * d],
                        start=(h == 0),
                        stop=(h == kh - 1),
                    )
                osb = sb.tile([128, d], f32, tag="osb")
                nc.vector.tensor_copy(out=osb[:], in_=pout[:])
                nc.scalar.dma_start(out[cl * 128 : (cl + 1) * 128, :], osb[:])

    nc.compile()
    return nc


# ----------------------------------------------------------------------------
# Entry point
# ----------------------------------------------------------------------------

_PROG_CACHE = {}


def _get_program(n_calls):
    if n_calls not in _PROG_CACHE:
        _PROG_CACHE[n_calls] = build_program(n_calls)
    return _PROG_CACHE[n_calls]


def make_in_maps(x, W, packs, n_calls):
    xb = np.ascontiguousarray(x.astype(BF16))
    wt = np.ascontiguousarray(W.T.reshape(D // 128, 128, D).astype(BF16))
    iota = np.broadcast_to(np.arange(W_RANK, dtype=np.float32), (128, W_RANK))
    iota = np.ascontiguousarray(iota.astype(BF16))
    ident = np.eye(128, dtype=np.float32).astype(BF16)
    in_maps = []
    for p in packs:
        idx, seg, val = build_call_arrays(p, n_calls)
        in_maps.append(
            dict(xb=xb, idx=idx, seg=seg, val=val, wt=wt, iota32=iota, ident=ident)
        )
    return in_maps


def kernel(x, W, edge_val, edge_row, edge_col, _return_results=False, trace=False):
    packs = pack_all(edge_row, edge_col, edge_val)
    n_calls = max(p["G"] // CG for p in packs)
    nc = _get_program(n_calls)
    in_maps = make_in_maps(x, W, packs, n_calls)
    res = run_bass_kernel_spmd(
        nc, in_maps, core_ids=list(range(N_CORES)), trace=trace
    )
    out = np.zeros((N_NODES, D), np.float32)
    for i, (p, core_out) in enumerate(zip(packs, res.results)):
        ov = core_out["out"]
        true_ids = p["dest"] + i * NPC
        if len(np.unique(true_ids)) == len(true_ids):
            out[true_ids] = ov[p["vrow"]]
        else:
            np.add.at(out, true_ids, ov[p["vrow"]])
    if _return_results:
        return out, res
    return out


# revision 16
# speedup vs baseline: 14165.3636x; 1.0250x over previous
"""GCN layer (linear + weighted scatter-add aggregation) on 8 TRN2 NeuronCores.

Reference computation:
    h = x @ W.T                      [N, D]
    out[r] = sum_{e: row[e]==r} val[e] * h[col[e]]

Key identity: the linear layer commutes past the (linear) aggregation:
    out = (A @ x) @ W.T    where A[r,c] = sum of val over edges (r,c)
so we aggregate raw x rows first (8x less matmul work, no h materialization).

Distribution: destination nodes are sharded 12500/core (edges partitioned by
destination so the segment-sum is fully local; x is replicated to each core's
HBM by the host, so no collective is needed).

Per-core algorithm ("perm-pack"):
  - Host packs *whole* destinations into "windows" of <=32 dests and 4x256
    edge slots, where the 4 quotas correspond to 4 source-node chunks of
    25000 rows (dma_gather indices are int16). The packing order defines a
    per-core virtual destination numbering; output rows are inverse-permuted
    (and summed, if a dest was split) on the host.
  - A window is 8 "groups" of 128 edge slots (2 per chunk). One "call" = 4
    windows = 32 groups = 4096 slots: four batched GPSIMD dma_gather ucode
    calls (1024 int16 indices each, one per source chunk) pull the x rows
    (bf16, 512B each) into SBUF as [128 slots, 32 groups, 256]. A banded
    scaled one-hot S ([128, 32] per group: S[p,r] = val[p] * (rank[p]==r))
    is built with 2 batched DVE ops per call.
  - PE: per group one matmul (lhsT = S band, rhs = gathered rows
    [128,256]) accumulating over the window's 8 groups into a 32-partition
    PSUM region; 4 windows fill 128 virtual dests' aggregate per call. The
    aggregate is cast to bf16, transposed on PE (identity trick), and
    multiplied by W.T (bf16, f32 PSUM) into the output block. No scatter,
    no atomics, no collectives.
"""

import os
import sys

sys.path.insert(0, "/opt/trn_rl_repo")
os.environ.setdefault("MYCRO_LOCAL_CACHE", "1")

from contextlib import ExitStack

import numpy as np
import ml_dtypes

import concourse.bass as bass
import concourse.bacc as bacc
import concourse.mybir as mybir
import concourse.tile as tile
from concourse.bass_utils import run_bass_kernel_spmd
from concourse.library_config import mlp as _mlp_lib

N_NODES = 100000
N_CORES = 8
NPC = N_NODES // N_CORES  # dests per core
D = 256
SLOTS = 128  # edge slots per group (= matmul K)
W_RANK = 32  # dests per window (= matmul M)
NCHUNK = 4
CHUNK = 25000  # source rows per chunk (int16-addressable)
W_CQ = 256  # window chunk quota (2 groups per chunk)
W_GROUPS = 8  # groups per window
W_SLOTS = W_GROUPS * SLOTS  # 1024 edge slots per window
CG = 32  # groups per call (4 windows -> 128 virtual dests/call)
WPC = 4  # windows per call
F_CALLS = 1  # calls fused per gather instruction (ring caps gathers at 1024 descriptors)
GATHER_IDX = F_CALLS * WPC * W_CQ  # 1024 indices per (call, chunk) dma_gather

BF16 = ml_dtypes.bfloat16


# ----------------------------------------------------------------------------
# Host-side packing
# ----------------------------------------------------------------------------

def pack_core(rows_loc, cols, vals, npc):
    """Pack one core's edges (dest-local ids in [0, npc)) into windows.

    Dests are placed greedily (alternating big/small by total degree) and may
    be SPLIT across consecutive windows when a per-chunk quota or the rank
    cap is hit, so windows fill to ~100%. Split partial sums are re-combined
    on the host (np.add.at over duplicate dest ids).

    Returns per-slot arrays idx (in gather slot order) and seg/val
    (in per-call slot order), plus per-item vrow/dest.
    """
    chunk_id = cols // CHUNK
    key = rows_loc.astype(np.int64) * NCHUNK + chunk_id
    order = np.argsort(key, kind="stable")
    cols_s = cols[order]
    vals_s = vals[order]
    dc_deg = np.bincount(key, minlength=npc * NCHUNK).astype(np.int64)
    dc_deg = dc_deg.reshape(npc, NCHUNK)
    dc_start = np.zeros(npc * NCHUNK + 1, np.int64)
    dc_start[1:] = np.cumsum(dc_deg.ravel())
    dc_start = dc_start[:-1].reshape(npc, NCHUNK)
    deg = dc_deg.sum(1)

    # alternating big/small feed order balances window sums
    srt = np.argsort(deg, kind="stable")
    feed = np.empty(npc, np.int64)
    feed[0::2] = srt[::-1][: (npc + 1) // 2]
    feed[1::2] = srt[: npc // 2]

    # greedy fill with splitting
    items_dest, items_w, items_rank = [], [], []
    items_take = []  # [NCHUNK] takes
    items_coff = []  # [NCHUNK] consumed offset within dest-chunk edges
    w = 0
    rank = 0
    rq = [W_CQ] * NCHUNK
    for d in feed:
        d = int(d)
        rem = dc_deg[d].copy()
        coff = np.zeros(NCHUNK, np.int64)
        while True:
            if rank == W_RANK:
                w += 1
                rank = 0
                rq = [W_CQ] * NCHUNK
            take = np.minimum(rem, rq)
            items_dest.append(d)
            items_w.append(w)
            items_rank.append(rank)
            items_take.append(take.copy())
            items_coff.append(coff.copy())
            rq = [int(rq[c] - take[c]) for c in range(NCHUNK)]
            rank += 1
            rem -= take
            coff += take
            if rem.sum() == 0:
                break
            # some chunk quota exhausted: close window, continue this dest
            w += 1
            rank = 0
            rq = [W_CQ] * NCHUNK
    n_windows = w + 1
    n_items = len(items_dest)
    items_dest = np.asarray(items_dest, np.int64)
    items_w = np.asarray(items_w, np.int64)
    items_rank = np.asarray(items_rank, np.int64)
    items_take = np.asarray(items_take, np.int64)  # [n_items, NCHUNK]
    items_coff = np.asarray(items_coff, np.int64)

    n_calls_local = (n_windows + WPC - 1) // WPC
    n_pairs = (n_calls_local + F_CALLS - 1) // F_CALLS
    n_calls_local = n_pairs * F_CALLS
    G = n_calls_local * CG

    # per-(item, chunk) slot bases in both slot orders
    call = items_w // WPC
    w_loc = items_w % WPC
    pair = call // F_CALLS
    q = call % F_CALLS
    # running offset within each window chunk quota
    qoff = np.zeros((n_items, NCHUNK), np.int64)
    cum = {}
    for i in range(n_items):
        ww = items_w[i]
        c0 = cum.get(ww)
        if c0 is None:
            c0 = np.zeros(NCHUNK, np.int64)
        qoff[i] = c0
        cum[ww] = c0 + items_take[i]
    # seg/val order: call*4096 + (8c + 2*w_loc)*128 + qoff
    base_sv = call * (CG * SLOTS)
    # idx order: pair*F*4096 + (8F*c + 8*q + 2*w_loc)*128 + qoff
    base_ix = pair * (F_CALLS * CG * SLOTS)

    flat_deg = items_take.ravel()
    cgrid = np.tile(np.arange(NCHUNK), n_items)
    irep = np.repeat(np.arange(n_items), NCHUNK)
    e_start = (dc_start[items_dest] + items_coff).ravel()
    sv_base = (
        base_sv[irep]
        + (8 * cgrid + 2 * w_loc[irep]) * SLOTS
        + qoff.ravel()
    )
    ix_base = (
        base_ix[irep]
        + (8 * F_CALLS * cgrid + 8 * q[irep] + 2 * w_loc[irep]) * SLOTS
        + qoff.ravel()
    )
    nz = np.nonzero(flat_deg)[0]
    nz_deg = flat_deg[nz]
    reps = np.repeat(np.arange(len(nz)), nz_deg)
    csum = np.zeros(len(nz) + 1, np.int64)
    csum[1:] = np.cumsum(nz_deg)
    within = np.arange(int(nz_deg.sum()), dtype=np.int64) - csum[reps]
    e_pos = e_start[nz][reps] + within
    slot_sv = sv_base[nz][reps] + within
    slot_ix = ix_base[nz][reps] + within

    idx_slot = np.zeros(G * SLOTS, np.int32)
    val_slot = np.zeros(G * SLOTS, np.float32)
    seg_slot = np.zeros(G * SLOTS, np.int16)
    idx_slot[slot_ix] = cols_s[e_pos] % CHUNK
    val_slot[slot_sv] = vals_s[e_pos]
    seg_slot[slot_sv] = items_rank[irep[nz]][reps]

    vrow = (items_w // WPC) * 128 + (items_w % WPC) * W_RANK + items_rank
    return dict(
        n_windows=n_windows,
        G=G,
        idx=idx_slot,
        val=val_slot,
        seg=seg_slot,
        vrow=vrow,
        dest=items_dest,
        n_edges=len(rows_loc),
    )


def pack_all(edge_row, edge_col, edge_val, n_nodes=N_NODES, n_cores=N_CORES):
    npc = n_nodes // n_cores
    core_id = edge_row // npc
    packs = []
    for i in range(n_cores):
        m = core_id == i
        packs.append(
            pack_core(edge_row[m] - i * npc, edge_col[m], edge_val[m], npc)
        )
    return packs


def build_call_arrays(p, n_calls):
    """DRAM layouts: seg/val [n_calls, 128, CG] (per-call slot order); idx
    wrapped int16 [n_pairs, NCHUNK, 128, GATHER_IDX//16] (pair-fused order).

    The dma_gather for (pair, chunk c) consumes the pair's 16 chunk-c groups
    in order; index position i -> (partition i%128, group 16c + i//128 of the
    pair tile), wrapped so position i sits at [i%16, i//16] (replicated over
    each 16-partition block).
    """
    G = p["G"]
    gtot = n_calls * CG
    n_pairs = n_calls // F_CALLS

    def lay(a, np_dtype):
        full = np.zeros(gtot * SLOTS, a.dtype)
        full[: G * SLOTS] = a
        return np.ascontiguousarray(
            full.reshape(n_calls, CG, SLOTS).transpose(0, 2, 1)
        ).astype(np_dtype)

    idx_full = np.zeros(gtot * SLOTS, np.int64)
    idx_full[: G * SLOTS] = p["idx"]
    byg = idx_full.reshape(n_pairs, F_CALLS * CG, SLOTS)
    iw = GATHER_IDX // 16
    idx = np.empty((n_pairs, NCHUNK, 128, iw), np.int16)
    gpc = 4 * F_CALLS * 2  # groups per chunk per pair (16)
    for c in range(NCHUNK):
        flat = byg[:, gpc * c : gpc * (c + 1), :].reshape(n_pairs, GATHER_IDX)
        wrapped = flat.reshape(n_pairs, iw, 16).transpose(0, 2, 1)
        idx[:, c, :, :] = np.tile(wrapped, (1, 8, 1)).astype(np.int16)

    return (
        np.ascontiguousarray(idx),
        lay(p["seg"], BF16),
        lay(p["val"], BF16),
    )


# ----------------------------------------------------------------------------
# Device program
# ----------------------------------------------------------------------------

def build_program(n_calls, n_nodes=N_NODES, d=D, chunk=CHUNK):
    nc = bacc.Bacc("TRN2", target_bir_lowering=False, debug=False)
    f32 = mybir.dt.float32
    bf16 = mybir.dt.bfloat16

    x = nc.dram_tensor("xb", [n_nodes, d], bf16, kind="ExternalInput")
    n_pairs = n_calls // F_CALLS
    iw = GATHER_IDX // 16  # idx words per chunk-gather per partition
    idxT = nc.dram_tensor(
        "idx", [n_pairs, NCHUNK, 128, iw], mybir.dt.int16, kind="ExternalInput"
    )
    segT = nc.dram_tensor("seg", [n_calls, SLOTS, CG], bf16, kind="ExternalInput")
    valT = nc.dram_tensor("val", [n_calls, SLOTS, CG], bf16, kind="ExternalInput")
    wtT = nc.dram_tensor("wt", [d // 128, 128, d], bf16, kind="ExternalInput")
    iotaT = nc.dram_tensor("iota32", [128, W_RANK], bf16, kind="ExternalInput")
    identT = nc.dram_tensor("ident", [128, 128], bf16, kind="ExternalInput")
    out = nc.dram_tensor("out", [n_calls * 128, d], bf16, kind="ExternalOutput")

    kh = d // 128  # feature half-tiles
    n_chunks = (n_nodes + chunk - 1) // chunk
    fcg = F_CALLS * CG  # groups per pair tile
    gpc = fcg // NCHUNK  # groups per chunk within a pair tile (16)

    with tile.TileContext(nc) as tc, ExitStack() as ctx:
        const = ctx.enter_context(tc.tile_pool(name="const", bufs=1))
        sb = ctx.enter_context(tc.tile_pool(name="sb", bufs=4))
        xgp = ctx.enter_context(tc.tile_pool(name="xg", bufs=3))
        ps = ctx.enter_context(tc.tile_pool(name="ps", bufs=2, space="PSUM"))

        nc.gpsimd.load_library(_mlp_lib)

        wt_t = const.tile([128, kh * d], bf16)
        for h in range(kh):
            nc.sync.dma_start(wt_t[:, h * d : (h + 1) * d], wtT[h])
        iota_t = const.tile([128, W_RANK], bf16)
        nc.sync.dma_start(iota_t[:], iotaT[:, :])
        id_t = const.tile([128, 128], bf16)
        nc.sync.dma_start(id_t[:], identT[:, :])

        for pr in range(n_pairs):
            idx_t = sb.tile([128, NCHUNK * iw], mybir.dt.int16, tag="idx")
            for c in range(NCHUNK):
                nc.sync.dma_start(idx_t[:, c * iw : (c + 1) * iw], idxT[pr, c])

            xg = xgp.tile([SLOTS, fcg, d], bf16, tag="xg")
            for c in range(min(n_chunks, NCHUNK)):
                lo = c * chunk
                hi = min(n_nodes, lo + chunk)
                nc.gpsimd.dma_gather(
                    xg[:, gpc * c : gpc * (c + 1), :],
                    x[lo:hi, :],
                    idx_t[:, c * iw : (c + 1) * iw],
                    GATHER_IDX,
                    GATHER_IDX,
                    d,
                )

            for q in range(F_CALLS):
                cl = pr * F_CALLS + q
                seg_t = sb.tile([SLOTS, CG], bf16, tag="seg")
                nc.sync.dma_start(seg_t[:], segT[cl])
                val_t = sb.tile([SLOTS, CG], bf16, tag="val")
                nc.sync.dma_start(val_t[:], valT[cl])

                # banded scaled one-hot: S[p, g, r] = val[p,g] * (seg[p,g] == r)
                d1 = sb.tile([SLOTS, CG, W_RANK], bf16, tag="d1")
                nc.vector.tensor_tensor(
                    out=d1[:],
                    in0=seg_t[:].unsqueeze(2).to_broadcast([SLOTS, CG, W_RANK]),
                    in1=iota_t[:].unsqueeze(1).to_broadcast([SLOTS, CG, W_RANK]),
                    op=mybir.AluOpType.subtract,
                )
                s_t = sb.tile([SLOTS, CG, W_RANK], bf16, tag="s")
                nc.vector.scalar_tensor_tensor(
                    out=s_t[:],
                    in0=d1[:],
                    scalar=0.0,
                    op0=mybir.AluOpType.is_equal,
                    in1=val_t[:].unsqueeze(2).to_broadcast([SLOTS, CG, W_RANK]),
                    op1=mybir.AluOpType.mult,
                )

                # base_partition() only supports 0/32/64, so two 64-partition
                # accumulators (windows 0,1 -> pacc_a; 2,3 -> pacc_b)
                pacc_a = ps.tile([64, d], f32, tag="pacc_a")
                pacc_b = ps.tile([64, d], f32, tag="pacc_b")
                for w_loc in range(WPC):
                    pacc = pacc_a if w_loc < 2 else pacc_b
                    off = (w_loc % 2) * W_RANK
                    for c in range(NCHUNK):
                        for j in range(2):
                            gq = 8 * c + 2 * w_loc + j  # per-call group (seg/val)
                            gt = gpc * c + 8 * q + 2 * w_loc + j  # pair-tile group
                            nc.tensor.matmul(
                                out=pacc[off : off + W_RANK, :],
                                lhsT=s_t[:, gq, :],
                                rhs=xg[:, gt, :],
                                start=(c == 0 and j == 0),
                                stop=(c == NCHUNK - 1 and j == 1),
                            )

                # cast aggregate to bf16, transpose on PE, multiply by W.T
                t1 = sb.tile([128, d], bf16, tag="t1")
                nc.vector.tensor_copy(out=t1[0:64, :], in_=pacc_a[:])
                nc.vector.tensor_copy(out=t1[64:128, :], in_=pacc_b[:])
                pT = ps.tile([128, kh * 128], bf16, tag="pT")
                for h in range(kh):
                    nc.tensor.transpose(
                        out=pT[:, h * 128 : (h + 1) * 128],
                        in_=t1[:, h * 128 : (h + 1) * 128],
                        identity=id_t[:],
                    )
                aggT = sb.tile([128, kh * 128], bf16, tag="aggT")
                nc.vector.tensor_copy(out=aggT[:], in_=pT[:])

                pout = ps.tile([128, d], f32, tag="pout")
                for h in range(kh):
                    nc.tensor.matmul(
                        out=pout[:],
                        lhsT=aggT[:, h * 128 : (h + 1) * 128],
                        rhs=wt_t[:, h * d : (h + 1) * d],
                        start=(h == 0),
                        stop=(h == kh - 1),
                    )
                osb = sb.tile([128, d], bf16, tag="osb")
                nc.vector.tensor_copy(out=osb[:], in_=pout[:])
                nc.scalar.dma_start(out[cl * 128 : (cl + 1) * 128, :], osb[:])

    nc.compile()
    return nc


# ----------------------------------------------------------------------------
# Entry point
# ----------------------------------------------------------------------------

_PROG_CACHE = {}


def _get_program(n_calls):
    if n_calls not in _PROG_CACHE:
        _PROG_CACHE[n_calls] = build_program(n_calls)
    return _PROG_CACHE[n_calls]


def make_in_maps(x, W, packs, n_calls):
    xb = np.ascontiguousarray(x.astype(BF16))
    wt = np.ascontiguousarray(W.T.reshape(D // 128, 128, D).astype(BF16))
    iota = np.broadcast_to(np.arange(W_RANK, dtype=np.float32), (128, W_RANK))
    iota = np.ascontiguousarray(iota.astype(BF16))
    ident = np.eye(128, dtype=np.float32).astype(BF16)
    in_maps = []
    for p in packs:
        idx, seg, val = build_call_arrays(p, n_calls)
        in_maps.append(
            dict(xb=xb, idx=idx, seg=seg, val=val, wt=wt, iota32=iota, ident=ident)
        )
    return in_maps


def kernel(x, W, edge_val, edge_row, edge_col, _return_results=False, trace=False):
    packs = pack_all(edge_row, edge_col, edge_val)
    n_calls = max(p["G"] // CG for p in packs)
    nc = _get_program(n_calls)
    in_maps = make_in_maps(x, W, packs, n_calls)
    res = run_bass_kernel_spmd(
        nc, in_maps, core_ids=list(range(N_CORES)), trace=trace
    )
    out = np.zeros((N_NODES, D), np.float32)
    for i, (p, core_out) in enumerate(zip(packs, res.results)):
        ov = np.asarray(core_out["out"]).astype(np.float32)
        true_ids = p["dest"] + i * NPC
        if len(np.unique(true_ids)) == len(true_ids):
            out[true_ids] = ov[p["vrow"]]
        else:
            np.add.at(out, true_ids, ov[p["vrow"]])
    if _return_results:
        return out, res
    return out


# revision 18
# speedup vs baseline: 14759.6767x; 1.0420x over previous
"""GCN layer (linear + weighted scatter-add aggregation) on 8 TRN2 NeuronCores.

Reference computation:
    h = x @ W.T                      [N, D]
    out[r] = sum_{e: row[e]==r} val[e] * h[col[e]]

Key identity: the linear layer commutes past the (linear) aggregation:
    out = (A @ x) @ W.T    where A[r,c] = sum of val over edges (r,c)
so we aggregate raw x rows first (8x less matmul work, no h materialization).

Distribution: destination nodes are sharded 12500/core (edges partitioned by
destination so the segment-sum is fully local; x is replicated to each core's
HBM by the host, so no collective is needed).

Per-core algorithm ("perm-pack"):
  - Host packs *whole* destinations into "windows" of <=32 dests and 4x256
    edge slots, where the 4 quotas correspond to 4 source-node chunks of
    25000 rows (dma_gather indices are int16). The packing order defines a
    per-core virtual destination numbering; output rows are inverse-permuted
    (and summed, if a dest was split) on the host.
  - A window is 8 "groups" of 128 edge slots (2 per chunk). One "call" = 4
    windows = 32 groups = 4096 slots: four batched GPSIMD dma_gather ucode
    calls (1024 int16 indices each, one per source chunk) pull the x rows
    (bf16, 512B each) into SBUF as [128 slots, 32 groups, 256]. A banded
    scaled one-hot S ([128, 32] per group: S[p,r] = val[p] * (rank[p]==r))
    is built with 2 batched DVE ops per call.
  - PE: per group one matmul (lhsT = S band, rhs = gathered rows
    [128,256]) accumulating over the window's 8 groups into a 32-partition
    PSUM region; 4 windows fill 128 virtual dests' aggregate per call. The
    aggregate is cast to bf16, transposed on PE (identity trick), and
    multiplied by W.T (bf16, f32 PSUM) into the output block. No scatter,
    no atomics, no collectives.
"""

import os
import sys

sys.path.insert(0, "/opt/trn_rl_repo")
os.environ.setdefault("MYCRO_LOCAL_CACHE", "1")

from contextlib import ExitStack

import numpy as np
import ml_dtypes

import concourse.bass as bass
import concourse.bacc as bacc
import concourse.mybir as mybir
import concourse.tile as tile
from concourse.bass_utils import run_bass_kernel_spmd
from concourse.library_config import mlp as _mlp_lib

N_NODES = 100000
N_CORES = 8
NPC = N_NODES // N_CORES  # dests per core
D = 256
SLOTS = 128  # edge slots per group (= matmul K)
W_RANK = 32  # dests per window (= matmul M)
NCHUNK = 4
CHUNK = 25000  # source rows per chunk (int16-addressable)
W_CQ = 256  # window chunk quota (2 groups per chunk)
W_GROUPS = 8  # groups per window
W_SLOTS = W_GROUPS * SLOTS  # 1024 edge slots per window
CG = 32  # groups per call (4 windows -> 128 virtual dests/call)
WPC = 4  # windows per call
F_CALLS = 1  # calls fused per gather instruction (ring caps gathers at 1024 descriptors)
GATHER_IDX = F_CALLS * WPC * W_CQ  # 1024 indices per (call, chunk) dma_gather

BF16 = ml_dtypes.bfloat16


# ----------------------------------------------------------------------------
# Host-side packing
# ----------------------------------------------------------------------------

def pack_core(rows_loc, cols, vals, npc):
    """Pack one core's edges (dest-local ids in [0, npc)) into windows.

    Dests are placed greedily (alternating big/small by total degree) and may
    be SPLIT across consecutive windows when a per-chunk quota or the rank
    cap is hit, so windows fill to ~100%. Split partial sums are re-combined
    on the host (np.add.at over duplicate dest ids).

    Returns per-slot arrays idx (in gather slot order) and seg/val
    (in per-call slot order), plus per-item vrow/dest.
    """
    chunk_id = cols // CHUNK
    key = rows_loc.astype(np.int64) * NCHUNK + chunk_id
    order = np.argsort(key, kind="stable")
    cols_s = cols[order]
    vals_s = vals[order]
    dc_deg = np.bincount(key, minlength=npc * NCHUNK).astype(np.int64)
    dc_deg = dc_deg.reshape(npc, NCHUNK)
    dc_start = np.zeros(npc * NCHUNK + 1, np.int64)
    dc_start[1:] = np.cumsum(dc_deg.ravel())
    dc_start = dc_start[:-1].reshape(npc, NCHUNK)
    deg = dc_deg.sum(1)

    # alternating big/small feed order balances window sums
    srt = np.argsort(deg, kind="stable")
    feed = np.empty(npc, np.int64)
    feed[0::2] = srt[::-1][: (npc + 1) // 2]
    feed[1::2] = srt[: npc // 2]

    # greedy fill with splitting
    items_dest, items_w, items_rank = [], [], []
    items_take = []  # [NCHUNK] takes
    items_coff = []  # [NCHUNK] consumed offset within dest-chunk edges
    w = 0
    rank = 0
    rq = [W_CQ] * NCHUNK
    for d in feed:
        d = int(d)
        rem = dc_deg[d].copy()
        coff = np.zeros(NCHUNK, np.int64)
        while True:
            if rank == W_RANK:
                w += 1
                rank = 0
                rq = [W_CQ] * NCHUNK
            take = np.minimum(rem, rq)
            items_dest.append(d)
            items_w.append(w)
            items_rank.append(rank)
            items_take.append(take.copy())
            items_coff.append(coff.copy())
            rq = [int(rq[c] - take[c]) for c in range(NCHUNK)]
            rank += 1
            rem -= take
            coff += take
            if rem.sum() == 0:
                break
            # some chunk quota exhausted: close window, continue this dest
            w += 1
            rank = 0
            rq = [W_CQ] * NCHUNK
    n_windows = w + 1
    n_items = len(items_dest)
    items_dest = np.asarray(items_dest, np.int64)
    items_w = np.asarray(items_w, np.int64)
    items_rank = np.asarray(items_rank, np.int64)
    items_take = np.asarray(items_take, np.int64)  # [n_items, NCHUNK]
    items_coff = np.asarray(items_coff, np.int64)

    n_calls_local = (n_windows + WPC - 1) // WPC
    n_pairs = (n_calls_local + F_CALLS - 1) // F_CALLS
    n_calls_local = n_pairs * F_CALLS
    G = n_calls_local * CG

    # per-(item, chunk) slot bases in both slot orders
    call = items_w // WPC
    w_loc = items_w % WPC
    pair = call // F_CALLS
    q = call % F_CALLS
    # running offset within each window chunk quota
    qoff = np.zeros((n_items, NCHUNK), np.int64)
    cum = {}
    for i in range(n_items):
        ww = items_w[i]
        c0 = cum.get(ww)
        if c0 is None:
            c0 = np.zeros(NCHUNK, np.int64)
        qoff[i] = c0
        cum[ww] = c0 + items_take[i]
    # seg/val order: call*4096 + (8c + 2*w_loc)*128 + qoff
    base_sv = call * (CG * SLOTS)
    # idx order: pair*F*4096 + (8F*c + 8*q + 2*w_loc)*128 + qoff
    base_ix = pair * (F_CALLS * CG * SLOTS)

    flat_deg = items_take.ravel()
    cgrid = np.tile(np.arange(NCHUNK), n_items)
    irep = np.repeat(np.arange(n_items), NCHUNK)
    e_start = (dc_start[items_dest] + items_coff).ravel()
    sv_base = (
        base_sv[irep]
        + (8 * cgrid + 2 * w_loc[irep]) * SLOTS
        + qoff.ravel()
    )
    ix_base = (
        base_ix[irep]
        + (8 * F_CALLS * cgrid + 8 * q[irep] + 2 * w_loc[irep]) * SLOTS
        + qoff.ravel()
    )
    nz = np.nonzero(flat_deg)[0]
    nz_deg = flat_deg[nz]
    reps = np.repeat(np.arange(len(nz)), nz_deg)
    csum = np.zeros(len(nz) + 1, np.int64)
    csum[1:] = np.cumsum(nz_deg)
    within = np.arange(int(nz_deg.sum()), dtype=np.int64) - csum[reps]
    e_pos = e_start[nz][reps] + within
    slot_sv = sv_base[nz][reps] + within
    slot_ix = ix_base[nz][reps] + within

    idx_slot = np.zeros(G * SLOTS, np.int32)
    val_slot = np.zeros(G * SLOTS, np.float32)
    seg_slot = np.zeros(G * SLOTS, np.int16)
    idx_slot[slot_ix] = cols_s[e_pos] % CHUNK
    val_slot[slot_sv] = vals_s[e_pos]
    seg_slot[slot_sv] = items_rank[irep[nz]][reps]

    vrow = (items_w // WPC) * 128 + (items_w % WPC) * W_RANK + items_rank
    return dict(
        n_windows=n_windows,
        G=G,
        idx=idx_slot,
        val=val_slot,
        seg=seg_slot,
        vrow=vrow,
        dest=items_dest,
        n_edges=len(rows_loc),
    )


def pack_all(edge_row, edge_col, edge_val, n_nodes=N_NODES, n_cores=N_CORES):
    npc = n_nodes // n_cores
    core_id = edge_row // npc
    packs = []
    for i in range(n_cores):
        m = core_id == i
        packs.append(
            pack_core(edge_row[m] - i * npc, edge_col[m], edge_val[m], npc)
        )
    return packs


def build_call_arrays(p, n_calls):
    """DRAM layouts: seg/val [n_calls, 128, CG] (per-call slot order); idx
    fused into one int16 meta tensor [n_calls, 128, 4*64+32+32]:
    4 chunk-gather index blocks (wrapped), then seg, then val (both bf16
    bit-packed).

    The dma_gather for (pair, chunk c) consumes the pair's 16 chunk-c groups
    in order; index position i -> (partition i%128, group 16c + i//128 of the
    pair tile), wrapped so position i sits at [i%16, i//16] (replicated over
    each 16-partition block).
    """
    G = p["G"]
    gtot = n_calls * CG
    n_pairs = n_calls // F_CALLS

    def lay(a, np_dtype):
        full = np.zeros(gtot * SLOTS, a.dtype)
        full[: G * SLOTS] = a
        return np.ascontiguousarray(
            full.reshape(n_calls, CG, SLOTS).transpose(0, 2, 1)
        ).astype(np_dtype)

    idx_full = np.zeros(gtot * SLOTS, np.int64)
    idx_full[: G * SLOTS] = p["idx"]
    byg = idx_full.reshape(n_pairs, F_CALLS * CG, SLOTS)
    iw = GATHER_IDX // 16
    meta = np.empty((n_calls, 128, NCHUNK * iw + 2 * CG), np.int16)
    gpc = 4 * F_CALLS * 2  # groups per chunk per gather tile
    for c in range(NCHUNK):
        flat = byg[:, gpc * c : gpc * (c + 1), :].reshape(n_pairs, GATHER_IDX)
        wrapped = flat.reshape(n_pairs, iw, 16).transpose(0, 2, 1)
        meta[:, :, c * iw : (c + 1) * iw] = np.tile(wrapped, (1, 8, 1)).astype(
            np.int16
        )
    o = NCHUNK * iw
    meta[:, :, o : o + CG] = lay(p["seg"], BF16).view(np.int16)
    meta[:, :, o + CG : o + 2 * CG] = lay(p["val"], BF16).view(np.int16)
    return np.ascontiguousarray(meta)


# ----------------------------------------------------------------------------
# Device program
# ----------------------------------------------------------------------------

def build_program(n_calls, n_nodes=N_NODES, d=D, chunk=CHUNK):
    nc = bacc.Bacc("TRN2", target_bir_lowering=False, debug=False)
    f32 = mybir.dt.float32
    bf16 = mybir.dt.bfloat16

    x = nc.dram_tensor("xb", [n_nodes, d], bf16, kind="ExternalInput")
    n_pairs = n_calls // F_CALLS
    iw = GATHER_IDX // 16  # idx words per chunk-gather per partition
    mw = NCHUNK * iw + 2 * CG  # meta words per partition per call
    metaT = nc.dram_tensor(
        "meta", [n_calls, 128, mw], mybir.dt.int16, kind="ExternalInput"
    )
    wtT = nc.dram_tensor("wt", [d // 128, 128, d], bf16, kind="ExternalInput")
    iotaT = nc.dram_tensor("iota32", [128, W_RANK], bf16, kind="ExternalInput")
    identT = nc.dram_tensor("ident", [128, 128], bf16, kind="ExternalInput")
    out = nc.dram_tensor("out", [n_calls * 128, d], bf16, kind="ExternalOutput")

    kh = d // 128  # feature half-tiles
    n_chunks = (n_nodes + chunk - 1) // chunk
    fcg = F_CALLS * CG  # groups per pair tile
    gpc = fcg // NCHUNK  # groups per chunk within a pair tile (16)

    with tile.TileContext(nc) as tc, ExitStack() as ctx:
        const = ctx.enter_context(tc.tile_pool(name="const", bufs=1))
        sb = ctx.enter_context(tc.tile_pool(name="sb", bufs=4))
        xgp = ctx.enter_context(tc.tile_pool(name="xg", bufs=3))
        ps = ctx.enter_context(tc.tile_pool(name="ps", bufs=2, space="PSUM"))

        nc.gpsimd.load_library(_mlp_lib)

        wt_t = const.tile([128, kh * d], bf16)
        for h in range(kh):
            nc.sync.dma_start(wt_t[:, h * d : (h + 1) * d], wtT[h])
        iota_t = const.tile([128, W_RANK], bf16)
        nc.sync.dma_start(iota_t[:], iotaT[:, :])
        id_t = const.tile([128, 128], bf16)
        nc.sync.dma_start(id_t[:], identT[:, :])

        for pr in range(n_pairs):
            idx_t = sb.tile([128, mw], mybir.dt.int16, tag="idx")
            nc.sync.dma_start(idx_t[:], metaT[pr])

            xg = xgp.tile([SLOTS, fcg, d], bf16, tag="xg")
            for c in range(min(n_chunks, NCHUNK)):
                lo = c * chunk
                hi = min(n_nodes, lo + chunk)
                nc.gpsimd.dma_gather(
                    xg[:, gpc * c : gpc * (c + 1), :],
                    x[lo:hi, :],
                    idx_t[:, c * iw : (c + 1) * iw],
                    GATHER_IDX,
                    GATHER_IDX,
                    d,
                )

            for q in range(F_CALLS):
                cl = pr * F_CALLS + q
                o = NCHUNK * iw
                seg_t = idx_t[:, o : o + CG].bitcast(bf16)
                val_t = idx_t[:, o + CG : o + 2 * CG].bitcast(bf16)

                # banded scaled one-hot: S[p, g, r] = val[p,g] * (seg[p,g] == r)
                d1 = sb.tile([SLOTS, CG, W_RANK], bf16, tag="d1")
                nc.vector.tensor_tensor(
                    out=d1[:],
                    in0=seg_t.unsqueeze(2).to_broadcast([SLOTS, CG, W_RANK]),
                    in1=iota_t[:].unsqueeze(1).to_broadcast([SLOTS, CG, W_RANK]),
                    op=mybir.AluOpType.subtract,
                )
                s_t = sb.tile([SLOTS, CG, W_RANK], bf16, tag="s")
                nc.vector.scalar_tensor_tensor(
                    out=s_t[:],
                    in0=d1[:],
                    scalar=0.0,
                    op0=mybir.AluOpType.is_equal,
                    in1=val_t.unsqueeze(2).to_broadcast([SLOTS, CG, W_RANK]),
                    op1=mybir.AluOpType.mult,
                )

                # base_partition() only supports 0/32/64, so two 64-partition
                # accumulators (windows 0,1 -> pacc_a; 2,3 -> pacc_b)
                pacc_a = ps.tile([64, d], f32, tag="pacc_a")
                pacc_b = ps.tile([64, d], f32, tag="pacc_b")
                for w_loc in range(WPC):
                    pacc = pacc_a if w_loc < 2 else pacc_b
                    off = (w_loc % 2) * W_RANK
                    for c in range(NCHUNK):
                        for j in range(2):
                            gq = 8 * c + 2 * w_loc + j  # per-call group (seg/val)
                            gt = gpc * c + 8 * q + 2 * w_loc + j  # pair-tile group
                            nc.tensor.matmul(
                                out=pacc[off : off + W_RANK, :],
                                lhsT=s_t[:, gq, :],
                                rhs=xg[:, gt, :],
                                start=(c == 0 and j == 0),
                                stop=(c == NCHUNK - 1 and j == 1),
                            )

                # cast aggregate to bf16, transpose on PE, multiply by W.T
                t1 = sb.tile([128, d], bf16, tag="t1")
                nc.vector.tensor_copy(out=t1[0:64, :], in_=pacc_a[:])
                nc.vector.tensor_copy(out=t1[64:128, :], in_=pacc_b[:])
                pT = ps.tile([128, kh * 128], bf16, tag="pT")
                for h in range(kh):
                    nc.tensor.transpose(
                        out=pT[:, h * 128 : (h + 1) * 128],
                        in_=t1[:, h * 128 : (h + 1) * 128],
                        identity=id_t[:],
                    )
                aggT = sb.tile([128, kh * 128], bf16, tag="aggT")
                nc.vector.tensor_copy(out=aggT[:], in_=pT[:])

                pout = ps.tile([128, d], f32, tag="pout")
                for h in range(kh):
                    nc.tensor.matmul(
                        out=pout[:],
                        lhsT=aggT[:, h * 128 : (h + 1) * 128],
                        rhs=wt_t[:, h * d : (h + 1) * d],
                        start=(h == 0),
                        stop=(h == kh - 1),
                    )
                osb = sb.tile([128, d], bf16, tag="osb")
                nc.vector.tensor_copy(out=osb[:], in_=pout[:])
                nc.scalar.dma_start(out[cl * 128 : (cl + 1) * 128, :], osb[:])

    nc.compile()
    return nc


# ----------------------------------------------------------------------------
# Entry point
# ----------------------------------------------------------------------------

_PROG_CACHE = {}


def _get_program(n_calls):
    if n_calls not in _PROG_CACHE:
        _PROG_CACHE[n_calls] = build_program(n_calls)
    return _PROG_CACHE[n_calls]


def make_in_maps(x, W, packs, n_calls):
    xb = np.ascontiguousarray(x.astype(BF16))
    wt = np.ascontiguousarray(W.T.reshape(D // 128, 128, D).astype(BF16))
    iota = np.broadcast_to(np.arange(W_RANK, dtype=np.float32), (128, W_RANK))
    iota = np.ascontiguousarray(iota.astype(BF16))
    ident = np.eye(128, dtype=np.float32).astype(BF16)
    in_maps = []
    for p in packs:
        meta = build_call_arrays(p, n_calls)
        in_maps.append(
            dict(xb=xb, meta=meta, wt=wt, iota32=iota, ident=ident)
        )
    return in_maps


def kernel(x, W, edge_val, edge_row, edge_col, _return_results=False, trace=False):
    packs = pack_all(edge_row, edge_col, edge_val)
    n_calls = max(p["G"] // CG for p in packs)
    nc = _get_program(n_calls)
    in_maps = make_in_maps(x, W, packs, n_calls)
    res = run_bass_kernel_spmd(
        nc, in_maps, core_ids=list(range(N_CORES)), trace=trace
    )
    out = np.zeros((N_NODES, D), np.float32)
    for i, (p, core_out) in enumerate(zip(packs, res.results)):
        ov = np.asarray(core_out["out"]).astype(np.float32)
        true_ids = p["dest"] + i * NPC
        if len(np.unique(true_ids)) == len(true_ids):
            out[true_ids] = ov[p["vrow"]]
        else:
            np.add.at(out, true_ids, ov[p["vrow"]])
    if _return_results:
        return out, res
    return out
